# revision 2
# baseline (speedup 1.0000x reference)
"""TRN2 Bass kernel for nn_CDF: out[i,j] = order[floor(ndtr(noise[i,j])*N), j].

Architecture (per NeuronCore, 8 cores, column-sharded 32 cols each):
  - No prep pass: dma_gather reads 512B blocks (4 rows x 32 cols) directly
    from the row-major order slice; block index s = idx>>2 fits int16 and a
    512B descriptor costs the same as 256B, so reblocking is pointless.
  - Per 128-row tile: index chain on ACT+DVE, PE transposes build the
    wrapped idx layout idxs[p, 8c+m] = s(row 16m+p, col c), then ONE PE
    matmul against a constant replication matrix broadcasts it to all
    eight 16-partition groups (no replication DMAs).
  - 4 x 1024-idx gathers per tile (ring-safe at nq=4) share one [128,4096]
    output tile; descriptor n = 128c'+i lands on partition i, slot c', so
    extraction is diagonal-AP (stride 129) copies: 1 plain + 3 predicated
    on lo2 masks, covering all 32 columns per op via a 2-dim free AP.
  - Extractions are software-pipelined 2 tile-steps behind the gathers so
    the in-order DVE stream never stalls on gather completion.
"""

import numpy as np

import concourse.bacc as bacc
import concourse.bass as bass
import concourse.mybir as mybir
import concourse.tile as tile
from concourse.bass_utils import run_bass_kernel_spmd
from concourse.masks import make_identity

N_CORES = 8
BATCH = 16384
N_DIM = 256
N_TRAIN = 100000
COLS = N_DIM // N_CORES          # 32 columns per core
P = 128

INV_SQRT2 = 0.7071067811865476

F32 = mybir.dt.float32
I32 = mybir.dt.int32
I16 = mybir.dt.int16
A = mybir.AluOpType


def build_nc(batch=BATCH, n_train=N_TRAIN, cols=COLS, nq=4, cpg=8,
             act_fn=None):
    assert n_train % 4 == 0
    nblk = n_train // 4              # 25000 512B blocks
    assert nblk - 1 <= 32767
    assert cols % cpg == 0
    n_cg = cols // cpg               # gathers per 128-row tile
    num_idxs = P * cpg               # idxs per gather
    rows_per_unit = 2048
    n_units = batch // rows_per_unit          # 8
    gathers_per_unit = rows_per_unit // P     # 16 (128-row tiles per unit)
    n_idx_tiles = 4

    nc = bacc.Bacc("TRN2", target_bir_lowering=False, debug=False,
                   num_swdge_queues=nq, dynamic_dma_scratch_size=2 ** 16)
    noise_d = nc.dram_tensor("noise", [batch, cols], F32, kind="ExternalInput")
    order_d = nc.dram_tensor("order", [n_train, cols], F32,
                             kind="ExternalInput")
    out_d = nc.dram_tensor("out", [batch, cols], F32, kind="ExternalOutput")

    oap = order_d.ap()
    # [nblk, 128] view of the order slice: block s = rows 4s..4s+3, 32 cols
    order_blocks = bass.AP(oap.tensor, oap.offset, [[128, nblk], [1, 128]])

    gq = [0]

    with tile.TileContext(nc) as tc:
        with tc.tile_pool(name="const", bufs=1) as cpool, \
             tc.tile_pool(name="io", bufs=2) as iopool, \
             tc.tile_pool(name="work", bufs=4) as wpool, \
             tc.tile_pool(name="xp", bufs=3) as xpool, \
             tc.tile_pool(name="gath", bufs=4) as gpool, \
             tc.tile_pool(name="ps1", bufs=2, space="PSUM") as ps1, \
             tc.tile_pool(name="ps2", bufs=2, space="PSUM") as ps2:

            ident = cpool.tile([P, P], F32)
            make_identity(nc, ident[:])

            # rep_ident[k, 16j+p] = delta(k, p): one matmul replicates a
            # 16-partition tile into all eight 16-partition groups.
            rep_ident = cpool.tile([16, P], F32, tag="repid")
            for j in range(8):
                nc.vector.tensor_copy(rep_ident[:, 16 * j:16 * j + 16],
                                      ident[:16, :16])

            idx_tiles = []
            for t in range(n_idx_tiles):
                it = cpool.tile([P, 256], I16, tag=f"idx{t}")
                nc.vector.memset(it[:], 0)
                idx_tiles.append(it)

            unit_state = {}

            def front(step):
                u, g = divmod(step, gathers_per_unit)
                if g == 0:
                    # load noise[2048u + 128rr + q, c] -> noise_t[q, 32rr+c]
                    noise_t = iopool.tile([P, 512], F32, tag="noise")
                    nap = noise_d.ap()
                    src_ap = bass.AP(
                        nap.tensor, nap.offset + u * rows_per_unit * cols,
                        [[cols, P], [P * cols, 16], [1, 32]],
                    )
                    nc.sync.dma_start(
                        noise_t[:].rearrange("p (rr c) -> p rr c", c=32),
                        src_ap)
                    acc_t = iopool.tile([P, 512], F32, tag="acc")
                    unit_state[u] = (noise_t, acc_t)
                noise_t, acc_t = unit_state[u]

                if True:
                    idxs_t = idx_tiles[step % n_idx_tiles]
                    x = noise_t[:, 32 * g:32 * g + 32]
                    e = wpool.tile([P, 32], F32, tag="e")
                    nc.scalar.activation(
                        e[:], x,
                        act_fn or mybir.ActivationFunctionType.Erf,
                        scale=INV_SQRT2)
                    tf = wpool.tile([P, 32], F32, tag="tf")
                    nc.vector.tensor_scalar(tf[:], e[:], 0.5 * n_train,
                                            0.5 * n_train - 0.5,
                                            A.mult, A.add)
                    ti = wpool.tile([P, 32], I32, tag="ti")
                    nc.vector.tensor_copy(ti[:], tf[:])
                    nc.vector.tensor_scalar(ti[:], ti[:], n_train - 1, 0,
                                            A.min, A.max)
                    lo2 = wpool.tile([P, 32], I32, tag="lo2")
                    nc.vector.tensor_scalar(lo2[:], ti[:], 3, None,
                                            A.bitwise_and)
                    s32 = wpool.tile([P, 32], I32, tag="s32")
                    nc.vector.tensor_scalar(s32[:], ti[:], 2, None,
                                            A.arith_shift_right)
                    sf = wpool.tile([P, 32], F32, tag="sf")
                    nc.vector.tensor_copy(sf[:], s32[:])

                    # T1: sf [128, 32] -> X [32, 128]
                    xps = ps1.tile([32, P], F32, tag="t1")
                    nc.tensor.transpose(xps[:], sf[:], ident[:])
                    X = xpool.tile([32, P], F32, tag="X")
                    nc.vector.tensor_copy(X[:], xps[:])

                    # stage[p, 8c + m] = s(row 128g+16m+p, col c); the
                    # per-gather slice [:, 64cg:64cg+64] is automatically
                    # the wrapped stream for columns cpg*cg..cpg*(cg+1).
                    stage = xpool.tile([16, 256], F32, tag="stage")
                    for m in range(8):
                        t2 = ps2.tile([16, 32], F32, tag="t2")
                        nc.tensor.transpose(t2[:], X[:, 16 * m:16 * m + 16],
                                            ident[:32, :32])
                        sv = stage[:]
                        dst = bass.AP(sv.tensor, sv.offset + m,
                                      [list(sv.ap[0][:1]) + [16],
                                       [8, 32]])
                        nc.vector.tensor_copy(dst, t2[:])
                    # replicate to all 8 q-groups in one matmul + convert
                    rep_ps = ps1.tile([P, 256], F32, tag="rep")
                    nc.tensor.matmul(rep_ps[:], rep_ident[:], stage[:],
                                     start=True, stop=True)
                    nc.vector.tensor_copy(idxs_t[:], rep_ps[:])

                    masks = []
                    for r in range(1, 4):
                        m = wpool.tile([P, 32], I32, tag=f"m{r}")
                        nc.vector.tensor_scalar(m[:], lo2[:], r, None,
                                                A.is_equal)
                        masks.append(m)

                    # ---- gathers: descriptor n = 128c' + i, one shared
                    #      output tile per step ----
                    g_big = gpool.tile([P, 4096], F32, tag="g")
                    for cg in range(n_cg):
                        out_sl = g_big[:, num_idxs * cg:num_idxs * (cg + 1)]
                        nc.gpsimd.dma_gather(
                            out_ap=out_sl.rearrange("p (n x) -> p n x",
                                                    x=128),
                            in_ap=order_blocks,
                            idxs_ap=idxs_t[:, 8 * cpg * cg:
                                           8 * cpg * cg + 8 * cpg],
                            num_idxs=num_idxs,
                            num_idxs_reg=num_idxs,
                            elem_size=128,
                            queue_num=gq[0] % nq,
                        )
                        gq[0] += 1
                    return g_big, masks, acc_t

            def back(step, g_big, masks, acc_t):
                u, g = divmod(step, gathers_per_unit)
                # elem (row 128g+q, col cpg*cg+c') at
                #   g_big[q, (num_idxs+cpg)*cg + 129c' + 32r]
                gv = g_big[:]
                av = acc_t[:]
                mstruct = [[cpg, n_cg], [1, cpg]]
                dst = bass.AP(av.tensor, av.offset + 32 * g,
                              [list(av.ap[0])] + mstruct)
                for r in range(4):
                    src = bass.AP(gv.tensor, gv.offset + 32 * r,
                                  [list(gv.ap[0]),
                                   [num_idxs + cpg, n_cg], [129, cpg]])
                    if r == 0:
                        nc.vector.tensor_copy(dst, src)
                    else:
                        mv = masks[r - 1][:]
                        mask_ap = bass.AP(mv.tensor, mv.offset,
                                          [list(mv.ap[0])] + mstruct)
                        nc.vector.copy_predicated(dst, mask_ap, src)
                if g == gathers_per_unit - 1:
                    # store acc_t[q, 32rr+c] -> out[2048u + 128rr + q, c]
                    odap = out_d.ap()
                    dst_ap = bass.AP(
                        odap.tensor, odap.offset + u * rows_per_unit * cols,
                        [[cols, P], [P * cols, 16], [1, 32]],
                    )
                    nc.sync.dma_start(
                        dst_ap,
                        acc_t[:].rearrange("p (rr c) -> p rr c", c=32))

            DEPTH = 2
            n_steps = n_units * gathers_per_unit
            pending = []
            for step in range(n_steps + DEPTH):
                if step < n_steps:
                    pending.append((step, front(step)))
                if step >= DEPTH:
                    bstep, (g_ts, masks, acc_t) = pending.pop(0)
                    back(bstep, g_ts, masks, acc_t)

    nc.compile()
    return nc


_nc_cache = {}


def _get_nc():
    if "nc" not in _nc_cache:
        _nc_cache["nc"] = build_nc()
    return _nc_cache["nc"]


def kernel(noise: np.ndarray, order: np.ndarray) -> np.ndarray:
    noise = np.ascontiguousarray(np.asarray(noise, dtype=np.float32))
    order = np.ascontiguousarray(np.asarray(order, dtype=np.float32))
    assert noise.shape == (BATCH, N_DIM)
    assert order.shape == (N_TRAIN, N_DIM)
    nc = _get_nc()
    in_maps = [
        {
            "noise": np.ascontiguousarray(noise[:, c * COLS:(c + 1) * COLS]),
            "order": np.ascontiguousarray(order[:, c * COLS:(c + 1) * COLS]),
        }
        for c in range(N_CORES)
    ]
    res = run_bass_kernel_spmd(nc, in_maps, core_ids=list(range(N_CORES)))
    return np.concatenate([r["out"] for r in res.results], axis=1)


# revision 3
# speedup vs baseline: 1.0019x; 1.0019x over previous
"""TRN2 Bass kernel for nn_CDF: out[i,j] = order[floor(ndtr(noise[i,j])*N), j].

Architecture (per NeuronCore, 8 cores, column-sharded 32 cols each):
  - No prep pass: dma_gather reads 512B blocks (4 rows x 32 cols) straight
    from the row-major order slice (block index s = idx>>2 fits int16, and
    512B descriptors price the same as 256B), so no table reblocking.
  - Per 128-row tile: erf/index chain on ACT+DVE, PE transposes build the
    SWDGE wrapped idx layout idxs[p, 8c+m] = s(row 16m+p, col c), and ONE
    PE matmul against a constant replication identity broadcasts it to all
    eight 16-partition groups (replaces per-gather replication DMAs).
  - 4 x 1024-idx gathers per tile (1024 descriptors/gather is a hard ucode
    ring limit) share one [128, 4096] output tile; descriptor n = 128c'+i
    lands on partition i, slot c', so extraction is 4 diagonal-AP ops
    (stride 129; 1 copy + 3 copy_predicated on lo2 masks) covering all 32
    columns each via 2-dim free APs.
  - Extractions are software-pipelined 2 tile-steps behind their gathers
    so the in-order DVE stream never blocks on gather completion; noise
    for the next 2048-row unit is prefetched a unit ahead.
"""

import numpy as np

import concourse.bacc as bacc
import concourse.bass as bass
import concourse.mybir as mybir
import concourse.tile as tile
from concourse.bass_utils import run_bass_kernel_spmd
from concourse.masks import make_identity

N_CORES = 8
BATCH = 16384
N_DIM = 256
N_TRAIN = 100000
COLS = N_DIM // N_CORES          # 32 columns per core
P = 128

INV_SQRT2 = 0.7071067811865476

F32 = mybir.dt.float32
I32 = mybir.dt.int32
I16 = mybir.dt.int16
A = mybir.AluOpType


def build_nc(batch=BATCH, n_train=N_TRAIN, cols=COLS, nq=4, cpg=8,
             act_fn=None):
    assert n_train % 4 == 0
    nblk = n_train // 4              # 25000 512B blocks
    assert nblk - 1 <= 32767
    assert cols % cpg == 0
    n_cg = cols // cpg               # gathers per 128-row tile
    num_idxs = P * cpg               # idxs per gather
    rows_per_unit = 2048
    n_units = batch // rows_per_unit          # 8
    gathers_per_unit = rows_per_unit // P     # 16 (128-row tiles per unit)
    n_idx_tiles = 4

    nc = bacc.Bacc("TRN2", target_bir_lowering=False, debug=False,
                   num_swdge_queues=nq, dynamic_dma_scratch_size=2 ** 16)
    noise_d = nc.dram_tensor("noise", [batch, cols], F32, kind="ExternalInput")
    order_d = nc.dram_tensor("order", [n_train, cols], F32,
                             kind="ExternalInput")
    out_d = nc.dram_tensor("out", [batch, cols], F32, kind="ExternalOutput")

    oap = order_d.ap()
    # [nblk, 128] view of the order slice: block s = rows 4s..4s+3, 32 cols
    order_blocks = bass.AP(oap.tensor, oap.offset, [[128, nblk], [1, 128]])

    gq = [0]

    with tile.TileContext(nc) as tc:
        with tc.tile_pool(name="const", bufs=1) as cpool, \
             tc.tile_pool(name="io", bufs=3) as iopool, \
             tc.tile_pool(name="work", bufs=4) as wpool, \
             tc.tile_pool(name="xp", bufs=3) as xpool, \
             tc.tile_pool(name="gath", bufs=5) as gpool, \
             tc.tile_pool(name="ps1", bufs=2, space="PSUM") as ps1, \
             tc.tile_pool(name="ps2", bufs=2, space="PSUM") as ps2:

            ident = cpool.tile([P, P], F32)
            make_identity(nc, ident[:])

            # rep_ident[k, 16j+p] = delta(k, p): one matmul replicates a
            # 16-partition tile into all eight 16-partition groups.
            rep_ident = cpool.tile([16, P], F32, tag="repid")
            for j in range(8):
                nc.vector.tensor_copy(rep_ident[:, 16 * j:16 * j + 16],
                                      ident[:16, :16])

            idx_tiles = []
            for t in range(n_idx_tiles):
                it = cpool.tile([P, 256], I16, tag=f"idx{t}")
                nc.vector.memset(it[:], 0)
                idx_tiles.append(it)

            nidx_reg = nc.gpsimd.to_reg(num_idxs)
            unit_state = {}

            def load_unit(u):
                # load noise[2048u + 128rr + q, c] -> noise_t[q, 32rr+c]
                noise_t = iopool.tile([P, 512], F32, tag="noise")
                nap = noise_d.ap()
                src_ap = bass.AP(
                    nap.tensor, nap.offset + u * rows_per_unit * cols,
                    [[cols, P], [P * cols, 16], [1, 32]],
                )
                nc.sync.dma_start(
                    noise_t[:].rearrange("p (rr c) -> p rr c", c=32),
                    src_ap)
                acc_t = iopool.tile([P, 512], F32, tag="acc")
                unit_state[u] = (noise_t, acc_t)

            load_unit(0)

            def front(step):
                u, g = divmod(step, gathers_per_unit)
                if g == 0 and u + 1 < n_units:
                    load_unit(u + 1)  # prefetch next unit's noise
                noise_t, acc_t = unit_state[u]

                if True:
                    idxs_t = idx_tiles[step % n_idx_tiles]
                    x = noise_t[:, 32 * g:32 * g + 32]
                    e = wpool.tile([P, 32], F32, tag="e")
                    nc.scalar.activation(
                        e[:], x,
                        act_fn or mybir.ActivationFunctionType.Erf,
                        scale=INV_SQRT2)
                    tf = wpool.tile([P, 32], F32, tag="tf")
                    nc.vector.tensor_scalar(tf[:], e[:], 0.5 * n_train,
                                            0.5 * n_train - 0.5,
                                            A.mult, A.add)
                    ti = wpool.tile([P, 32], I32, tag="ti")
                    nc.vector.tensor_copy(ti[:], tf[:])
                    nc.vector.tensor_scalar(ti[:], ti[:], n_train - 1, 0,
                                            A.min, A.max)
                    lo2 = wpool.tile([P, 32], I32, tag="lo2")
                    nc.vector.tensor_scalar(lo2[:], ti[:], 3, None,
                                            A.bitwise_and)
                    s32 = wpool.tile([P, 32], I32, tag="s32")
                    nc.vector.tensor_scalar(s32[:], ti[:], 2, None,
                                            A.arith_shift_right)
                    sf = wpool.tile([P, 32], F32, tag="sf")
                    nc.vector.tensor_copy(sf[:], s32[:])

                    # T1: sf [128, 32] -> X [32, 128]
                    xps = ps1.tile([32, P], F32, tag="t1")
                    nc.tensor.transpose(xps[:], sf[:], ident[:])
                    X = xpool.tile([32, P], F32, tag="X")
                    nc.vector.tensor_copy(X[:], xps[:])

                    # stage[p, 8c + m] = s(row 128g+16m+p, col c); the
                    # per-gather slice [:, 64cg:64cg+64] is automatically
                    # the wrapped stream for columns cpg*cg..cpg*(cg+1).
                    stage = xpool.tile([16, 256], F32, tag="stage")
                    for m in range(8):
                        t2 = ps2.tile([16, 32], F32, tag="t2")
                        nc.tensor.transpose(t2[:], X[:, 16 * m:16 * m + 16],
                                            ident[:32, :32])
                        sv = stage[:]
                        dst = bass.AP(sv.tensor, sv.offset + m,
                                      [list(sv.ap[0][:1]) + [16],
                                       [8, 32]])
                        nc.vector.tensor_copy(dst, t2[:])
                    # replicate to all 8 q-groups in one matmul + convert
                    rep_ps = ps1.tile([P, 256], F32, tag="rep")
                    nc.tensor.matmul(rep_ps[:], rep_ident[:], stage[:],
                                     start=True, stop=True)
                    nc.vector.tensor_copy(idxs_t[:], rep_ps[:])

                    masks = []
                    for r in range(1, 4):
                        m = wpool.tile([P, 32], I32, tag=f"m{r}")
                        nc.vector.tensor_scalar(m[:], lo2[:], r, None,
                                                A.is_equal)
                        masks.append(m)

                    # ---- gathers: descriptor n = 128c' + i, one shared
                    #      output tile per step ----
                    g_big = gpool.tile([P, 4096], F32, tag="g")
                    for cg in range(n_cg):
                        out_sl = g_big[:, num_idxs * cg:num_idxs * (cg + 1)]
                        nc.gpsimd.dma_gather(
                            out_ap=out_sl.rearrange("p (n x) -> p n x",
                                                    x=128),
                            in_ap=order_blocks,
                            idxs_ap=idxs_t[:, 8 * cpg * cg:
                                           8 * cpg * cg + 8 * cpg],
                            num_idxs=num_idxs,
                            num_idxs_reg=nidx_reg,
                            elem_size=128,
                            queue_num=gq[0] % nq,
                        )
                        gq[0] += 1
                    return g_big, masks, acc_t

            def back(step, g_big, masks, acc_t):
                u, g = divmod(step, gathers_per_unit)
                # elem (row 128g+q, col cpg*cg+c') at
                #   g_big[q, (num_idxs+cpg)*cg + 129c' + 32r]
                gv = g_big[:]
                av = acc_t[:]
                mstruct = [[cpg, n_cg], [1, cpg]]
                dst = bass.AP(av.tensor, av.offset + 32 * g,
                              [list(av.ap[0])] + mstruct)
                for r in range(4):
                    src = bass.AP(gv.tensor, gv.offset + 32 * r,
                                  [list(gv.ap[0]),
                                   [num_idxs + cpg, n_cg], [129, cpg]])
                    if r == 0:
                        nc.vector.tensor_copy(dst, src)
                    else:
                        mv = masks[r - 1][:]
                        mask_ap = bass.AP(mv.tensor, mv.offset,
                                          [list(mv.ap[0])] + mstruct)
                        nc.vector.copy_predicated(dst, mask_ap, src)
                if g == gathers_per_unit - 1:
                    # store acc_t[q, 32rr+c] -> out[2048u + 128rr + q, c]
                    odap = out_d.ap()
                    dst_ap = bass.AP(
                        odap.tensor, odap.offset + u * rows_per_unit * cols,
                        [[cols, P], [P * cols, 16], [1, 32]],
                    )
                    nc.sync.dma_start(
                        dst_ap,
                        acc_t[:].rearrange("p (rr c) -> p rr c", c=32))

            DEPTH = 2
            n_steps = n_units * gathers_per_unit
            pending = []
            for step in range(n_steps + DEPTH):
                if step < n_steps:
                    pending.append((step, front(step)))
                if step >= DEPTH:
                    bstep, (g_ts, masks, acc_t) = pending.pop(0)
                    back(bstep, g_ts, masks, acc_t)

    nc.compile()
    return nc


_nc_cache = {}


def _get_nc():
    if "nc" not in _nc_cache:
        _nc_cache["nc"] = build_nc()
    return _nc_cache["nc"]


def kernel(noise: np.ndarray, order: np.ndarray) -> np.ndarray:
    noise = np.ascontiguousarray(np.asarray(noise, dtype=np.float32))
    order = np.ascontiguousarray(np.asarray(order, dtype=np.float32))
    assert noise.shape == (BATCH, N_DIM)
    assert order.shape == (N_TRAIN, N_DIM)
    nc = _get_nc()
    in_maps = [
        {
            "noise": np.ascontiguousarray(noise[:, c * COLS:(c + 1) * COLS]),
            "order": np.ascontiguousarray(order[:, c * COLS:(c + 1) * COLS]),
        }
        for c in range(N_CORES)
    ]
    res = run_bass_kernel_spmd(nc, in_maps, core_ids=list(range(N_CORES)))
    return np.concatenate([r["out"] for r in res.results], axis=1)
